# revision 8
# baseline (speedup 1.0000x reference)
"""Trainium2 Bass kernel for nn_AllAtomDecoder (gnn_message_passing).

Math: all 34 side-chain atom slots of residue i are placed at CA_i, so the
[A,A] (A = L*34) radius-graph adjacency is a residue-level [L,L] adjacency
R expanded by per-atom validity vm:
    adj[(i,s),(j,t)] = R[i,j] * vm[i,s] * vm[j,t] * (1 - delta_{(i,s),(j,t)})
with R[i,i] = 1 (distance 0 < 8).  Hence
    msg[(i,s),:] = vm[i,s] * (M[i,:] - remb[i,:] - atom_sc[s,:])
where S[j,:] = cnt_j * remb[j,:] + vm[j,:] @ atom_sc   (cnt_j = sum_t vm[j,t])
      M     = R @ S                                    ([L,L] @ [L,D])
With W = tbl_sc @ [atom_sc | 1] ([20, D+1]):  [temb | cnt] = onehotT.T @ W,
so the only PE work is: one-hot transposes, W, onehot@W, scm4, and R@S.
Pairwise distances are computed directly on the vector engine as
sum_c (ca_j[c] - ca_own[c])^2 against a host-broadcast row block.

Sharding: 8 cores; cores 0-3 own batch 0, cores 4-7 batch 1; each core
computes the residue-level stages for its batch and emits 32 residues
([32, 34*128] f32) of the final output.

Implementation: raw bacc (no TileContext) with hand-placed semaphores.
The per-core output [32 res, 34 t, 128 d] is computed with the t-axis
split into 4 groups (9,9,8,8) packed onto the partition axis:
partition p = 32*tg + l, so the two big DVE ops run ~1.2k columns on
all 128 partitions.  All small inputs ride in one packed [128, 708]
tensor, DMA'd as two halves on the two HWDGE rings (sync + scalar).
"""

from contextlib import ExitStack

import numpy as np

import concourse.bass as bass
import concourse.bacc as bacc
import concourse.mybir as mybir
from concourse.bass_utils import run_bass_kernel_spmd

F32 = mybir.dt.float32
ALU = mybir.AluOpType
AX = mybir.AxisListType

B = 2
L = 128          # residues per batch
NCLS = 20        # enabled residue classes (>=20 are argmax-disabled)
NSC = 34         # side-chain atom slots
D = 128          # embedding dim
RPC = 32         # residues per core
NCORES = 8
R2 = 64.0        # RADIUS**2

TB = [0, 9, 18, 26]   # t-group bases
TW = [9, 9, 8, 8]     # t-group widths

_widths = dict(aa_f=NCLS, remb_f=D, maskv=1, tblT=NCLS, atomones=D + 1,
               aa_o4=NCLS, remb_o4=D, mask_o4=1, eye=L, catT=3, cat_ob=96,
               tbl=NSC)
_off = {}
_c = 0
for _name, _w in _widths.items():
    _off[_name] = _c
    _c += _w
PACKW = _c  # 708


def build_nc():
    """Build the SPMD per-core Bass graph (identical on all 8 cores)."""
    nc = bacc.Bacc("TRN2", target_bir_lowering=False, debug=False,
                   num_devices=NCORES)

    pack = nc.dram_tensor("pack", [L, PACKW], F32, kind="ExternalInput")
    atom = nc.dram_tensor("atom", [NSC, D], F32, kind="ExternalInput")
    out = nc.dram_tensor("out", [RPC, NSC * D], F32, kind="ExternalOutput")
    out3 = out[:].rearrange("l (t d) -> l t d", d=D)
    aflat = atom[:].rearrange("t d -> (t d)")

    with ExitStack() as ctx:
        e = ctx.enter_context

        # ---------------- SBUF ----------------
        pk = e(nc.sbuf_tensor([L, PACKW], F32))
        oh_all = e(nc.sbuf_tensor([L, 64], F32))
        rmax_f = e(nc.sbuf_tensor([L, 1], F32))
        rmax_o = e(nc.sbuf_tensor([L, 1], F32))
        dx = e(nc.sbuf_tensor([L, 96], F32))
        sq96 = e(nc.sbuf_tensor([L, 96], F32))
        s01 = e(nc.sbuf_tensor([L, RPC], F32))
        acc = e(nc.sbuf_tensor([L, RPC], F32))
        ohT_f = e(nc.sbuf_tensor([RPC, L], F32))
        ohT_o4 = e(nc.sbuf_tensor([RPC, L], F32))
        W2 = e(nc.sbuf_tensor([NCLS, D + 1], F32))
        S_t = e(nc.sbuf_tensor([L, D], F32))
        rcols4 = e(nc.sbuf_tensor([L, L], F32))
        q4 = e(nc.sbuf_tensor([L, D], F32))
        atom_rep = e(nc.sbuf_tensor([L, 9, D], F32))
        v4 = e(nc.sbuf_tensor([L, 9, D], F32))
        o4 = e(nc.sbuf_tensor([L, 9, D], F32))

        def pv(name, rows=L):
            return pk[:rows, _off[name]:_off[name] + _widths[name]]

        aa_f_t = pv("aa_f")
        remb_f_t = pv("remb_f")
        maskv_t = pv("maskv")
        tblT_t = pv("tblT", NSC)
        atomones_t = pv("atomones", NSC)
        aa_o4_t = pv("aa_o4")
        remb_o4_t = pv("remb_o4")
        mask_o4_t = pv("mask_o4")
        eye_t = pv("eye")
        catT_t = pv("catT")
        cat_ob_t = pv("cat_ob")
        tbl_t = pv("tbl", NCLS)

        # ---------------- PSUM (5 banks) ----------------
        ohT_f_p = e(nc.psum_tensor([RPC, L], F32))       # b0
        ohT_o4_p = e(nc.psum_tensor([RPC, L], F32))      # b1
        w2m4_p = e(nc.psum_tensor([L, D + 1], F32))      # b2: W2 rows 0:20 -> m4
        temb2_p = e(nc.psum_tensor([L, D + 1], F32))     # b3
        scm4_p = e(nc.psum_tensor([L, 9], F32))          # b4

        sem_in = e(nc.semaphore("sem_in"))
        sem_atom = e(nc.semaphore("sem_atom"))
        sem_dve = e(nc.semaphore("sem_dve"))
        sem_pe = e(nc.semaphore("sem_pe"))
        sem_out = e(nc.semaphore("sem_out"))

        block = e(nc.Block())

        # ---------------- DMA ring 1: sync ----------------
        @block.sync
        def _(eng):
            eng.dma_start(pk[:64, :], pack[:64, :]).then_inc(sem_in, 16)
            eng.wait_ge(sem_dve, 20)            # mul_a done -> tg2/tg3 ready
            for tg in (2, 3):
                eng.dma_start(
                    out3[:, TB[tg]:TB[tg] + TW[tg], :],
                    o4[32 * tg:32 * (tg + 1), :TW[tg], :],
                ).then_inc(sem_out, 16)
            eng.wait_ge(sem_out, 64)            # all output landed

        # ---------------- DMA ring 2: scalar ----------------
        @block.scalar
        def _(eng):
            eng.dma_start(pk[64:, :], pack[64:, :]).then_inc(sem_in, 16)
            for tg in range(4):
                tb, tw = TB[tg], TW[tg]
                eng.dma_start(
                    atom_rep[32 * tg:32 * (tg + 1), :tw, :]
                    .rearrange("l t d -> l (t d)"),
                    aflat[tb * D:(tb + tw) * D][None, :]
                    .to_broadcast((RPC, tw * D)),
                ).then_inc(sem_atom, 16)
            eng.wait_ge(sem_dve, 21)            # mul_b done -> tg0/tg1 ready
            for tg in (0, 1):
                eng.dma_start(
                    out3[:, TB[tg]:TB[tg] + TW[tg], :],
                    o4[32 * tg:32 * (tg + 1), :TW[tg], :],
                ).then_inc(sem_out, 16)

        # ---------------- DVE ----------------
        @block.vector
        def _(eng):
            v = nc.vector
            v.memset(oh_all[:], 0.0).then_inc(sem_dve, 1)           # 1
            eng.wait_ge(sem_in, 32)
            v.tensor_reduce(rmax_f[:], aa_f_t, op=ALU.max,
                            axis=AX.X).then_inc(sem_dve, 1)         # 2
            v.tensor_reduce(rmax_o[:], aa_o4_t, op=ALU.max,
                            axis=AX.X).then_inc(sem_dve, 1)         # 3
            eng.wait_ge(sem_dve, 3)
            v.tensor_scalar(oh_all[:, :NCLS], aa_f_t, rmax_f[:, :1],
                            maskv_t, ALU.is_ge,
                            ALU.mult).then_inc(sem_dve, 1)          # 4
            v.tensor_scalar(oh_all[:, 32:32 + NCLS], aa_o4_t,
                            rmax_o[:, :1], mask_o4_t, ALU.is_ge,
                            ALU.mult).then_inc(sem_dve, 1)          # 5
            # distances on DVE: dx_c = cat_own_bcast_c - ca_j[c]
            for c in range(3):
                v.tensor_scalar(dx[:, 32 * c:32 * (c + 1)],
                                cat_ob_t[:, 32 * c:32 * (c + 1)],
                                catT_t[:, c:c + 1], None,
                                ALU.subtract).then_inc(sem_dve, 1)  # 6-8
            eng.wait_ge(sem_dve, 8)
            v.tensor_tensor(sq96[:], dx[:], dx[:],
                            op=ALU.mult).then_inc(sem_dve, 1)       # 9
            eng.wait_ge(sem_dve, 9)
            v.tensor_tensor(s01[:], sq96[:, :RPC], sq96[:, RPC:2 * RPC],
                            op=ALU.add).then_inc(sem_dve, 1)        # 10
            eng.wait_ge(sem_dve, 10)
            v.tensor_tensor(acc[:], s01[:], sq96[:, 2 * RPC:],
                            op=ALU.add).then_inc(sem_dve, 1)        # 11
            eng.wait_ge(sem_dve, 11)
            v.tensor_scalar(
                rcols4[:].rearrange("j (a b) -> j a b", b=RPC),
                acc[:, None, :].to_broadcast((L, 4, RPC)),
                R2, None, ALU.is_lt).then_inc(sem_dve, 1)           # 12
            eng.wait_ge(sem_pe, 3)              # transposes done
            v.tensor_copy(ohT_f[:], ohT_f_p[:]).then_inc(sem_dve, 1)     # 13
            v.tensor_copy(ohT_o4[:], ohT_o4_p[:]).then_inc(sem_dve, 1)   # 14
            v.tensor_copy(W2[:], w2m4_p[:NCLS, :]).then_inc(sem_dve, 1)  # 15
            eng.wait_ge(sem_pe, 4)              # temb2 done
            v.scalar_tensor_tensor(S_t[:], remb_f_t, temb2_p[:, D:D + 1],
                                   temb2_p[:, :D], ALU.mult,
                                   ALU.add).then_inc(sem_dve, 1)    # 16: S
            eng.wait_ge(sem_pe, 9)              # m4 done
            v.tensor_tensor(q4[:], w2m4_p[:, :D], remb_o4_t,
                            op=ALU.subtract).then_inc(sem_dve, 1)   # 17: q4
            eng.wait_ge(sem_atom, 64)
            eng.wait_ge(sem_dve, 17)
            v.tensor_tensor(
                v4[:, :8, :], q4[:, None, :].to_broadcast((L, 8, D)),
                atom_rep[:, :8, :],
                op=ALU.subtract).then_inc(sem_dve, 1)               # 18 sub_a
            v.tensor_tensor(
                v4[:64, 8:9, :], q4[:64, None, :].to_broadcast((64, 1, D)),
                atom_rep[:64, 8:9, :],
                op=ALU.subtract).then_inc(sem_dve, 1)               # 19 sub_b
            eng.wait_ge(sem_dve, 19)
            v.tensor_tensor(
                o4[:, :8, :], v4[:, :8, :],
                scm4_p[:, :8, None].to_broadcast((L, 8, D)),
                op=ALU.mult).then_inc(sem_dve, 1)                   # 20 mul_a
            v.tensor_tensor(
                o4[:64, 8:9, :], v4[:64, 8:9, :],
                scm4_p[:64, 8:9, None].to_broadcast((64, 1, D)),
                op=ALU.mult).then_inc(sem_dve, 1)                   # 21 mul_b

        # ---------------- PE ----------------
        @block.tensor
        def _(eng):
            t = nc.tensor
            eng.wait_ge(sem_in, 32)
            t.matmul(w2m4_p[:NCLS, :], tblT_t,
                     atomones_t).then_inc(sem_pe, 1)                # 1: W2
            eng.wait_ge(sem_dve, 5)             # one-hots written
            t.transpose(ohT_f_p[:], oh_all[:, :RPC],
                        eye_t).then_inc(sem_pe, 1)                  # 2
            t.transpose(ohT_o4_p[:], oh_all[:, RPC:],
                        eye_t).then_inc(sem_pe, 1)                  # 3
            eng.wait_ge(sem_dve, 15)            # ohT + W2 copies done
            t.matmul(temb2_p[:], ohT_f[:NCLS, :],
                     W2[:]).then_inc(sem_pe, 1)                     # 4: [temb|cnt]
            for tg in range(4):
                tb, tw = TB[tg], TW[tg]
                t.matmul(scm4_p[32 * tg:32 * (tg + 1), :tw],
                         ohT_o4[:NCLS, 32 * tg:32 * (tg + 1)],
                         tbl_t[:, tb:tb + tw],
                         tile_position=(0, 32 * tg),
                         ).then_inc(sem_pe, 1)                      # 5-8
            eng.wait_ge(sem_dve, 16)            # rcols4 + S ready
            t.matmul(w2m4_p[:, :D], rcols4[:],
                     S_t[:]).then_inc(sem_pe, 1)                    # 9: m4

    nc.compile()
    return nc


def make_in_maps(aa_pred, residue_embeddings, bb_pred, mask,
                 valid_atom37_mask, atom_embed):
    f32 = lambda x: np.ascontiguousarray(x, dtype=np.float32)
    eye = np.eye(L, dtype=np.float32)
    tbl_sc = f32(valid_atom37_mask[:NCLS, 3:])          # [20, 34]
    atom_sc = f32(atom_embed[3:])                       # [34, 128]
    atomones = np.concatenate(
        [atom_sc, np.ones((NSC, 1), np.float32)], axis=1)
    in_maps = []
    for c in range(NCORES):
        b = c // (NCORES // B)
        r0 = (c % (NCORES // B)) * RPC
        pk = np.zeros((L, PACKW), dtype=np.float32)

        def put(name, arr):
            arr = f32(arr)
            pk[:arr.shape[0], _off[name]:_off[name] + arr.shape[1]] = arr

        put("aa_f", aa_pred[b, :, :NCLS])
        put("remb_f", residue_embeddings[b])
        put("maskv", mask[b][:, None])
        put("tblT", tbl_sc.T)
        put("atomones", atomones)
        put("aa_o4", np.tile(aa_pred[b, r0:r0 + RPC, :NCLS], (4, 1)))
        put("remb_o4", np.tile(residue_embeddings[b, r0:r0 + RPC], (4, 1)))
        put("mask_o4", np.tile(mask[b, r0:r0 + RPC][:, None], (4, 1)))
        put("eye", eye)
        put("catT", bb_pred[b, :, 1, :])
        put("cat_ob", np.tile(
            f32(bb_pred[b, r0:r0 + RPC, 1, :]).T.reshape(1, -1), (L, 1)))
        put("tbl", tbl_sc)
        in_maps.append({"pack": pk, "atom": atom_sc})
    return in_maps


def gather_out(results):
    chunks = [np.asarray(r["out"]).reshape(RPC, NSC, D) for r in results]
    full = np.concatenate(chunks, axis=0)          # [256, 34, 128]
    return full.reshape(B, L * NSC, D)


def kernel(**inputs) -> np.ndarray:
    nc = build_nc()
    in_maps = make_in_maps(**inputs)
    res = run_bass_kernel_spmd(nc, in_maps, core_ids=list(range(NCORES)))
    return gather_out(res.results)


# revision 29
# speedup vs baseline: 1.3104x; 1.3104x over previous
"""Trainium2 Bass kernel for nn_AllAtomDecoder (gnn_message_passing).

Math: all 34 side-chain atom slots of residue i are placed at CA_i, so the
[A,A] (A = L*34) radius-graph adjacency is a residue-level [L,L] adjacency
R expanded by per-atom validity vm:
    adj[(i,s),(j,t)] = R[i,j] * vm[i,s] * vm[j,t] * (1 - delta_{(i,s),(j,t)})
with R[i,i] = 1 (distance 0 < 8).  Hence
    msg[(i,s),:] = vm[i,s] * (M[i,:] - remb[i,:] - atom_sc[s,:])
where S[j,:] = cnt_j * remb[j,:] + vm[j,:] @ atom_sc   (cnt_j = sum_t vm[j,t])
      M     = R @ S                                    ([L,L] @ [L,D])
With W = tbl_sc @ [atom_sc | 1] ([20, D+1]):  [temb | cnt] = onehot @ W,
so the only PE work is: W, two one-hot transposes, onehot@W, scm4, R@S.
Pairwise distances run on the vector engine as sum_c (ca_j - ca_own)^2
against a host-broadcast row block; the is_lt(64) also expands R columns
4x along partitions for the t-grouped output layout.

Sharding: 8 cores; cores 0-3 own batch 0, cores 4-7 batch 1; each core
computes the residue-level stages for its batch and emits 32 residues
([32, 34*128] f32) of the final output.

Implementation: raw bacc (no TileContext), hand-placed semaphores, two
HWDGE rings (sync + scalar).  The [32 res, 34 t, 128 d] output is packed
as partition p = 32*tg + l over 4 t-groups (9,9,8,8) so the two big DVE
ops run ~1.2k columns on all 128 partitions.  bf16 (exact for one-hot /
table data) makes the PE matmuls single-pass.
"""

from contextlib import ExitStack

import ml_dtypes
import numpy as np

import concourse.bass as bass
import concourse.bacc as bacc
import concourse.mybir as mybir
from concourse.bass_utils import run_bass_kernel_spmd

F32 = mybir.dt.float32
BF16 = mybir.dt.bfloat16
ALU = mybir.AluOpType
AX = mybir.AxisListType

B = 2
L = 128          # residues per batch
NCLS = 20        # enabled residue classes (>=20 are argmax-disabled)
NSC = 34         # side-chain atom slots
D = 128          # embedding dim
RPC = 32         # residues per core
NCORES = 8
R2 = 64.0        # RADIUS**2
WAIT_OUT = False  # False: let the NEFF epilogue shadow the output-DMA drain

TB = [0, 9, 17, 25]   # t-group bases (tg1/tg2 and tg2/tg3 overlap by one
TW = [9, 9, 9, 9]     # column; duplicated columns compute identical bytes)

# pack column layout (f32 columns; aa2/mask2 pairs must stay adjacent)
_widths = dict(aa2=2 * NCLS, mask2=2, remb_f=D, tblT=NCLS, atomones=D + 1,
               remb_o4=D, eye=L, catT=3, cat_ob=96, tblbf=NSC // 2)
_off = {}
_c = 0
for _name, _w in _widths.items():
    _off[_name] = _c
    _c += _w
PACKW = _c


def build_nc():
    """Build the SPMD per-core Bass graph (identical on all 8 cores)."""
    nc = bacc.Bacc("TRN2", target_bir_lowering=False, debug=False,
                   num_devices=NCORES)

    pack = nc.dram_tensor("pack", [L, PACKW], F32, kind="ExternalInput")
    atom = nc.dram_tensor("atom", [NSC, D], BF16, kind="ExternalInput")
    out = nc.dram_tensor("out", [RPC, NSC * D], F32, kind="ExternalOutput")
    out3 = out[:].rearrange("l (t d) -> l t d", d=D)
    aflat = atom[:].rearrange("t d -> (t d)")

    with ExitStack() as ctx:
        e = ctx.enter_context

        # ---------------- SBUF ----------------
        pk = e(nc.sbuf_tensor([L, PACKW], F32))
        oh_all = e(nc.sbuf_tensor([L, 64], BF16))
        ohtmp = e(nc.sbuf_tensor([L, 2, NCLS], F32))
        rmax2 = e(nc.sbuf_tensor([L, 2], F32))
        dx = e(nc.sbuf_tensor([L, 96], F32))
        sq96 = e(nc.sbuf_tensor([L, 96], F32))
        s01 = e(nc.sbuf_tensor([L, RPC], F32))
        acc = e(nc.sbuf_tensor([L, RPC], F32))
        ohT2 = e(nc.sbuf_tensor([RPC, 2 * L], BF16))
        W2 = e(nc.sbuf_tensor([NCLS, D + 1], BF16))
        S_t = e(nc.sbuf_tensor([L, D], BF16))
        rcols4 = e(nc.sbuf_tensor([L, L], BF16))
        scm4s = e(nc.sbuf_tensor([L, 9], BF16))
        q4 = e(nc.sbuf_tensor([L, D], BF16))
        atom_rep = e(nc.sbuf_tensor([L, 9, D], BF16))
        v4 = e(nc.sbuf_tensor([L, 9, D], BF16))
        o4 = e(nc.sbuf_tensor([L, 9, D], F32))

        def pv(name, rows=L):
            return pk[:rows, _off[name]:_off[name] + _widths[name]]

        aa2_t = pv("aa2").rearrange("p (g c) -> p g c", g=2)       # [128,2,20]
        mask2_t = pv("mask2")                                      # [128,2]
        remb_f_t = pv("rembbf").bitcast(BF16)
        tblTbf_t = pv("tblTbf", NSC).bitcast(BF16)              # [34, 20]
        atonesbf_t = pv("atonesbf", NSC).bitcast(BF16)[:, :D + 1]  # [34, 129]
        remb_o4_t = pv("rembo4bf").bitcast(BF16)
        eye_t = pv("eye")
        catT_t = pv("catT")
        cat_ob_t = pv("cat_ob").rearrange("p (c m) -> p c m", c=3)
        tblbf_t = pv("tblbf", NCLS).bitcast(BF16)                  # [20,34] bf16
        # one-hot output view: groups at oh_all cols [0:20] and [32:52]
        oh_view = oh_all[:].rearrange("p (g c) -> p g c", g=2)[:, :, :NCLS]

        # ---------------- PSUM (4 banks) ----------------
        ohT2_p = e(nc.psum_tensor([RPC, 2 * L], BF16))   # b0: both transposes
        w2m4_p = e(nc.psum_tensor([L, D + 1], F32))      # b1: W2 rows 0:20 -> m4
        temb2_p = e(nc.psum_tensor([L, D + 1], F32))     # b2
        scm4_p = e(nc.psum_tensor([L, 9], F32))          # b3

        sem_in = e(nc.semaphore("sem_in"))
        sem_atom = e(nc.semaphore("sem_atom"))
        sem_atomg = e(nc.semaphore("sem_atomg"))
        sem_outg = e(nc.semaphore("sem_outg"))
        sem_dve = e(nc.semaphore("sem_dve"))
        sem_pe = e(nc.semaphore("sem_pe"))
        sem_out = e(nc.semaphore("sem_out"))

        block = e(nc.Block(no_gpsimd_drain=True))

        # ---------------- DMA ring 1: sync ----------------
        @block.sync
        def _(eng):
            eng.dma_start(warm[:, :8], pack[:1, :8]).then_inc(sem_warm, 16)
            eng.dma_start(pk[:64, :], pack[:64, :]).then_inc(sem_in, 16)
            eng.wait_ge(sem_dve, 15)            # mul_a done -> tg2/tg3 ready
            for tg in (2, 3):
                eng.dma_start(
                    out3[:, TB[tg]:TB[tg] + TW[tg], :],
                    o4[32 * tg:32 * (tg + 1), :TW[tg], :],
                ).then_inc(sem_out, 16)
            if WAIT_OUT:
                eng.wait_ge(sem_out, 64)        # all output landed

        # ---------------- DMA ring 2: scalar ----------------
        @block.scalar
        def _(eng):
            eng.dma_start(warm[:, 8:], pack[:1, 8:16]).then_inc(sem_warm, 16)
            eng.dma_start(pk[64:, :], pack[64:, :]).then_inc(sem_in, 16)
            for tg in range(4):
                tb, tw = TB[tg], TW[tg]
                eng.dma_start(
                    atom_rep[32 * tg:32 * (tg + 1), :tw, :]
                    .rearrange("l t d -> l (t d)"),
                    aflat[tb * D:(tb + tw) * D][None, :]
                    .to_broadcast((RPC, tw * D)),
                ).then_inc(sem_atom, 16)
            eng.wait_ge(sem_dve, 16)            # mul_b done -> tg0/tg1 ready
            for tg in (0, 1):
                eng.dma_start(
                    out3[:, TB[tg]:TB[tg] + TW[tg], :],
                    o4[32 * tg:32 * (tg + 1), :TW[tg], :],
                ).then_inc(sem_out, 16)

        # ---------------- DVE ----------------
        @block.vector
        def _(eng):
            v = nc.vector
            v.memset(oh_all[:], 0.0).then_inc(sem_dve, 1)           # 1
            eng.wait_ge(sem_g, 1)
            v.tensor_scalar(eye_bf[:], eye_sb[:], 0.0, None,
                            ALU.is_equal).then_inc(sem_dve, 1)      # 2: eye
            eng.wait_ge(sem_in, 32)
            v.tensor_reduce(rmax2[:, :, None], aa2_t, op=ALU.max,
                            axis=AX.X).then_inc(sem_dve, 1)         # 3
            eng.wait_ge(sem_dve, 3)
            v.tensor_tensor(ohtmp[:], aa2_t,
                            rmax2[:, :, None].to_broadcast((L, 2, NCLS)),
                            op=ALU.is_ge).then_inc(sem_dve, 1)      # 4
            eng.wait_ge(sem_dve, 4)
            v.tensor_tensor(oh_view, ohtmp[:],
                            mask2_t[:, :, None].to_broadcast((L, 2, NCLS)),
                            op=ALU.mult).then_inc(sem_dve, 1)       # 5
            eng.wait_ge(sem_pe, 2)              # transposes done
            v.tensor_copy(ohT2[:], ohT2_p[:]).then_inc(sem_dve, 1)      # 6
            eng.wait_ge(sem_pe, 3)              # W2 done
            v.tensor_copy(W2[:], w2m4_p[:NCLS, :]).then_inc(sem_dve, 1)  # 7
            eng.wait_ge(sem_d2, 4)              # distances ready
            v.tensor_scalar(
                rcols4[:].rearrange("j (a b) -> j a b", b=RPC),
                acc[:, None, :].to_broadcast((L, 4, RPC)),
                R2, None, ALU.is_lt).then_inc(sem_dve, 1)           # 8
            eng.wait_ge(sem_pe, 4)              # temb2 done
            eng.wait_ge(sem_cold2, 16)          # remb rows 64:128
            v.scalar_tensor_tensor(S_t[:], remb_f_t, temb2_p[:, D:D + 1],
                                   temb2_p[:, :D], ALU.mult,
                                   ALU.add).then_inc(sem_dve, 1)    # 9: S
            eng.wait_ge(sem_pe, 8)              # scm4 matmuls done
            v.tensor_copy(scm4s[:], scm4_p[:]).then_inc(sem_dve, 1)     # 10
            eng.wait_ge(sem_pe, 9)              # m4 done
            v.tensor_tensor(q4[:], w2m4_p[:, :D], remb_o4_t,
                            op=ALU.subtract).then_inc(sem_dve, 1)   # 11: q4
            eng.wait_ge(sem_atomg, 64)
            eng.wait_ge(sem_dve, 11)
            v.tensor_tensor(
                v4[:], q4[:, None, :].to_broadcast((L, 9, D)),
                atom_rep[:],
                op=ALU.subtract).then_inc(sem_dve, 1)               # 12 sub
            eng.wait_ge(sem_dve, 12)
            v.tensor_tensor(
                o4[:], v4[:],
                scm4s[:, :, None].to_broadcast((L, 9, D)),
                op=ALU.mult).then_inc(sem_dve, 1)                   # 13 mul

        # ---------------- PE ----------------
        @block.tensor
        def _(eng):
            t = nc.tensor
            eng.wait_ge(sem_dve, 5)             # one-hots written
            t.transpose(ohT2_p[:, :L], oh_all[:, :RPC],
                        eye_bf[:]).then_inc(sem_pe, 1)              # 1
            t.transpose(ohT2_p[:, L:], oh_all[:, RPC:],
                        eye_bf[:]).then_inc(sem_pe, 1)              # 2
            eng.wait_ge(sem_cold, 16)           # tables are in rows 0:64
            t.matmul(w2m4_p[:NCLS, :], tblTbf_t,
                     atonesbf_t).then_inc(sem_pe, 1)                # 3: W2
            eng.wait_ge(sem_dve, 7)             # ohT2 + W2 copies done
            t.matmul(temb2_p[:], ohT2[:NCLS, :L],
                     W2[:]).then_inc(sem_pe, 1)                     # 4: [temb|cnt]
            for tg in range(4):
                tb, tw = TB[tg], TW[tg]
                t.matmul(scm4_p[32 * tg:32 * (tg + 1), :tw],
                         ohT2[:NCLS, L + 32 * tg:L + 32 * (tg + 1)],
                         tblbf_t[:, tb:tb + tw],
                         tile_position=(0, 32 * tg),
                         ).then_inc(sem_pe, 1)                      # 5-8
            eng.wait_ge(sem_dve, 9)             # S + rcols4 ready
            t.matmul(w2m4_p[:, :D], rcols4[:],
                     S_t[:]).then_inc(sem_pe, 1)                    # 9: m4

    nc.compile()
    return nc


def make_in_maps(aa_pred, residue_embeddings, bb_pred, mask,
                 valid_atom37_mask, atom_embed):
    f32 = lambda x: np.ascontiguousarray(x, dtype=np.float32)
    eye = np.eye(L, dtype=np.float32)
    tbl_sc = f32(valid_atom37_mask[:NCLS, 3:])          # [20, 34]
    atom_sc = f32(atom_embed[3:])                       # [34, 128]
    atomones = np.concatenate(
        [atom_sc, np.ones((NSC, 1), np.float32)], axis=1)
    tblbf = np.ascontiguousarray(
        tbl_sc.astype(ml_dtypes.bfloat16)).view(np.float32)  # [20, 17]
    in_maps = []
    for c in range(NCORES):
        b = c // (NCORES // B)
        r0 = (c % (NCORES // B)) * RPC
        pk = np.zeros((L, PACKW), dtype=np.float32)

        def put(name, arr):
            arr = f32(arr)
            pk[:arr.shape[0], _off[name]:_off[name] + arr.shape[1]] = arr

        put("aa2", np.concatenate(
            [aa_pred[b, :, :NCLS],
             np.tile(aa_pred[b, r0:r0 + RPC, :NCLS], (4, 1))], axis=1))
        put("mask2", np.stack(
            [mask[b], np.tile(mask[b, r0:r0 + RPC], 4)], axis=1))
        put("rembbf", np.ascontiguousarray(
            f32(residue_embeddings[b]).astype(ml_dtypes.bfloat16))
            .view(np.float32))
        put("tblTbf", np.ascontiguousarray(
            tbl_sc.T.astype(ml_dtypes.bfloat16)).view(np.float32))
        aob = np.zeros((NSC, 130), ml_dtypes.bfloat16)
        aob[:, :D + 1] = atomones.astype(ml_dtypes.bfloat16)
        put("atonesbf", np.ascontiguousarray(aob).view(np.float32))
        put("rembo4bf", np.ascontiguousarray(np.tile(
            f32(residue_embeddings[b, r0:r0 + RPC]), (4, 1))
            .astype(ml_dtypes.bfloat16)).view(np.float32))
        put("eye", eye)
        put("catT", bb_pred[b, :, 1, :])
        put("cat_ob", np.tile(
            f32(bb_pred[b, r0:r0 + RPC, 1, :]).T.reshape(1, -1), (L, 1)))
        put("tblbf", tblbf)
        in_maps.append({"pack": pk,
                        "atom": atom_sc.astype(ml_dtypes.bfloat16)})
    return in_maps


def gather_out(results):
    chunks = [np.asarray(r["out"]).reshape(RPC, NSC, D) for r in results]
    full = np.concatenate(chunks, axis=0)          # [256, 34, 128]
    return full.reshape(B, L * NSC, D)


def kernel(**inputs) -> np.ndarray:
    nc = build_nc()
    in_maps = make_in_maps(**inputs)
    res = run_bass_kernel_spmd(nc, in_maps, core_ids=list(range(NCORES)))
    return gather_out(res.results)


# revision 31
# speedup vs baseline: 1.3129x; 1.0019x over previous
"""Trainium2 Bass kernel for nn_AllAtomDecoder (gnn_message_passing).

Math: all 34 side-chain atom slots of residue i are placed at CA_i, so the
[A,A] (A = L*34) radius-graph adjacency is a residue-level [L,L] adjacency
R expanded by per-atom validity vm:
    adj[(i,s),(j,t)] = R[i,j] * vm[i,s] * vm[j,t] * (1 - delta_{(i,s),(j,t)})
with R[i,i] = 1 (distance 0 < 8).  Hence
    msg[(i,s),:] = vm[i,s] * (M[i,:] - remb[i,:] - atom_sc[s,:])
where S[j,:] = cnt_j * remb[j,:] + vm[j,:] @ atom_sc   (cnt_j = sum_t vm[j,t])
      M     = R @ S                                    ([L,L] @ [L,D])
With W = tbl_sc @ [atom_sc | 1] ([20, D+1]):  [temb | cnt] = onehot @ W,
so the only PE work is: W, two one-hot transposes, onehot@W, scm4, R@S.
Pairwise distances run on the vector engine as sum_c (ca_j - ca_own)^2
against a host-broadcast row block; the is_lt(64) also expands R columns
4x along partitions for the t-grouped output layout.

Sharding: 8 cores; cores 0-3 own batch 0, cores 4-7 batch 1; each core
computes the residue-level stages for its batch and emits 32 residues
([32, 34*128] f32) of the final output.

Implementation: raw bacc (no TileContext), hand-placed semaphores.  The
[32 res, 34 t, 128 d] output is packed as partition p = 32*tg + l over 4
overlapping t-groups (bases 0/9/17/25, width 9; duplicated columns
compute identical bytes) so the two big DVE ops run 1152 columns on all
128 partitions.  bf16 (exact for one-hot / table data, ~1e-3 rel for
embeddings) makes the PE matmuls single-pass and the big subtract 2x.
Inputs ride in one packed tensor split hot/cold across the two HWDGE
rings (sync + scalar); the atom-embedding broadcast and the distance
chain run on the otherwise idle GPSIMD engine; output DMAs are spread
over all three DMA queues, and their drain is shadowed by the NEFF
epilogue (no engine waits on them; the epilogue outlasts the transfer).
"""

from contextlib import ExitStack

import ml_dtypes
import numpy as np

import concourse.bacc as bacc
import concourse.mybir as mybir
from concourse.bass_utils import run_bass_kernel_spmd

F32 = mybir.dt.float32
BF16 = mybir.dt.bfloat16
ALU = mybir.AluOpType
AX = mybir.AxisListType

B = 2
L = 128          # residues per batch
NCLS = 20        # enabled residue classes (>=20 are argmax-disabled)
NSC = 34         # side-chain atom slots
D = 128          # embedding dim
RPC = 32         # residues per core
NCORES = 8
R2 = 64.0        # RADIUS**2

TB = [0, 9, 17, 25]   # t-group bases (tg1/tg2 and tg2/tg3 overlap by one
TW = [9, 9, 9, 9]     # column; duplicated columns compute identical bytes)

# pack column layout (f32 columns; aa2/mask2 pairs must stay adjacent)
_widths = dict(aa2=2 * NCLS, mask2=2, catT=3, cat_ob=96,       # hot
               rembbf=D // 2, tblTbf=NCLS // 2, atonesbf=65,    # cold
               rembo4bf=D // 2, tblbf=NSC // 2)
HOTW = 2 * NCLS + 2 + 3 + 96
_off = {}
_c = 0
for _name, _w in _widths.items():
    _off[_name] = _c
    _c += _w
PACKW = _c


def build_nc():
    """Build the SPMD per-core Bass graph (identical on all 8 cores)."""
    nc = bacc.Bacc("TRN2", target_bir_lowering=False, debug=False,
                   num_devices=NCORES)

    pack = nc.dram_tensor("pack", [L, PACKW], F32, kind="ExternalInput")
    atom = nc.dram_tensor("atom", [NSC, D], BF16, kind="ExternalInput")
    out = nc.dram_tensor("out", [RPC, NSC * D], F32, kind="ExternalOutput")
    out3 = out[:].rearrange("l (t d) -> l t d", d=D)
    aflat = atom[:].rearrange("t d -> (t d)")

    with ExitStack() as ctx:
        e = ctx.enter_context

        # ---------------- SBUF ----------------
        pk = e(nc.sbuf_tensor([L, PACKW], F32))
        eye_sb = e(nc.sbuf_tensor([L, L], F32))
        eye_bf = e(nc.sbuf_tensor([L, L], BF16))
        oh_all = e(nc.sbuf_tensor([L, 64], BF16))
        ohtmp = e(nc.sbuf_tensor([L, 2, NCLS], F32))
        rmax2 = e(nc.sbuf_tensor([L, 2], F32))
        dx = e(nc.sbuf_tensor([L, 96], F32))
        sq96 = e(nc.sbuf_tensor([L, 96], F32))
        s01 = e(nc.sbuf_tensor([L, RPC], F32))
        acc = e(nc.sbuf_tensor([L, RPC], F32))
        ohT2 = e(nc.sbuf_tensor([RPC, 2 * L], BF16))
        W2 = e(nc.sbuf_tensor([NCLS, D + 1], BF16))
        S_t = e(nc.sbuf_tensor([L, D], BF16))
        rcols4 = e(nc.sbuf_tensor([L, L], BF16))
        scm4s = e(nc.sbuf_tensor([L, 9], BF16))
        q4 = e(nc.sbuf_tensor([L, D], BF16))
        atom_rep = e(nc.sbuf_tensor([L, 9, D], BF16))
        v4 = e(nc.sbuf_tensor([L, 9, D], BF16))
        o4 = e(nc.sbuf_tensor([L, 9, D], F32))

        def pv(name, rows=L):
            return pk[:rows, _off[name]:_off[name] + _widths[name]]


        aa2_t = pv("aa2").rearrange("p (g c) -> p g c", g=2)       # [128,2,20]
        mask2_t = pv("mask2")                                      # [128,2]
        remb_f_t = pv("rembbf").bitcast(BF16)
        tblTbf_t = pv("tblTbf", NSC).bitcast(BF16)              # [34, 20]
        atonesbf_t = pv("atonesbf", NSC).bitcast(BF16)[:, :D + 1]  # [34, 129]
        remb_o4_t = pv("rembo4bf").bitcast(BF16)
        catT_t = pv("catT")
        cat_ob_t = pv("cat_ob").rearrange("p (c m) -> p c m", c=3)
        tblbf_t = pv("tblbf", NCLS).bitcast(BF16)                  # [20,34] bf16
        # one-hot output view: groups at oh_all cols [0:20] and [32:52]
        oh_view = oh_all[:].rearrange("p (g c) -> p g c", g=2)[:, :, :NCLS]

        # ---------------- PSUM (4 banks) ----------------
        ohT2_p = e(nc.psum_tensor([RPC, 2 * L], BF16))   # b0: both transposes
        w2m4_p = e(nc.psum_tensor([L, D + 1], F32))      # b1: W2 rows 0:20 -> m4
        temb2_p = e(nc.psum_tensor([L, D + 1], F32))     # b2
        scm4_p = e(nc.psum_tensor([L, 9], F32))          # b3

        sem_g = e(nc.semaphore("sem_g"))
        sem_d2 = e(nc.semaphore("sem_d2"))
        sem_cold = e(nc.semaphore("sem_cold"))
        sem_cold2 = e(nc.semaphore("sem_cold2"))
        sem_in = e(nc.semaphore("sem_in"))
        sem_atomg = e(nc.semaphore("sem_atomg"))
        sem_outg = e(nc.semaphore("sem_outg"))
        sem_dve = e(nc.semaphore("sem_dve"))
        sem_pe = e(nc.semaphore("sem_pe"))
        sem_out = e(nc.semaphore("sem_out"))

        block = e(nc.Block(no_gpsimd_drain=True))

        # ------- GPSIMD: identity iota, 4 atom DMAs, distance chain -------
        @block.gpsimd
        def _(eng):
            g = nc.gpsimd
            g.iota(eye_sb[:], pattern=[[1, L]], base=0,
                   channel_multiplier=-1,
                   allow_small_or_imprecise_dtypes=True).then_inc(sem_g, 1)
            for tg in range(4):
                tb, tw = TB[tg], TW[tg]
                g.dma_start(
                    atom_rep[32 * tg:32 * (tg + 1), :tw, :]
                    .rearrange("l t d -> l (t d)"),
                    aflat[tb * D:(tb + tw) * D][None, :]
                    .to_broadcast((RPC, tw * D)),
                ).then_inc(sem_atomg, 16)
            eng.wait_ge(sem_in, 32)
            g.tensor_tensor(dx[:].rearrange("p (c m) -> p c m", c=3),
                            cat_ob_t,
                            catT_t[:, :, None].to_broadcast((L, 3, RPC)),
                            op=ALU.subtract).then_inc(sem_d2, 1)    # 1
            eng.wait_ge(sem_d2, 1)
            g.tensor_tensor(sq96[:], dx[:], dx[:],
                            op=ALU.mult).then_inc(sem_d2, 1)        # 2
            eng.wait_ge(sem_d2, 2)
            g.tensor_tensor(s01[:], sq96[:, :RPC], sq96[:, RPC:2 * RPC],
                            op=ALU.add).then_inc(sem_d2, 1)         # 3
            eng.wait_ge(sem_d2, 3)
            g.tensor_tensor(acc[:], s01[:], sq96[:, 2 * RPC:],
                            op=ALU.add).then_inc(sem_d2, 1)         # 4
            eng.wait_ge(sem_dve, 13)            # mul done
            g.dma_start(
                out3[:, TB[3]:TB[3] + 9, :],
                o4[96:, :, :],
            ).then_inc(sem_outg, 16)

        # ---------------- DMA ring 1: sync ----------------
        @block.sync
        def _(eng):
            eng.dma_start(pk[:64, :HOTW], pack[:64, :HOTW]).then_inc(sem_in, 16)
            eng.dma_start(pk[:64, HOTW:], pack[:64, HOTW:]).then_inc(sem_cold, 16)
            eng.wait_ge(sem_dve, 13)            # mul done
            eng.dma_start(
                out3[:, TB[2]:TB[2] + 9, :],
                o4[64:96, :, :],
            ).then_inc(sem_out, 16)


        # ---------------- DMA ring 2: scalar ----------------
        @block.scalar
        def _(eng):
            eng.dma_start(pk[64:, :HOTW], pack[64:, :HOTW]).then_inc(sem_in, 16)
            eng.dma_start(pk[64:, HOTW:], pack[64:, HOTW:]).then_inc(sem_cold2, 16)
            eng.wait_ge(sem_dve, 13)            # mul done
            eng.dma_start(
                out3[:, :2 * 9, :].rearrange("l (g w) d -> g l (w d)", g=2),
                o4[:64, :, :],
            ).then_inc(sem_out, 16)


        # ---------------- DVE ----------------
        @block.vector
        def _(eng):
            v = nc.vector
            v.memset(oh_all[:], 0.0).then_inc(sem_dve, 1)           # 1
            eng.wait_ge(sem_g, 1)
            v.tensor_scalar(eye_bf[:], eye_sb[:], 0.0, None,
                            ALU.is_equal).then_inc(sem_dve, 1)      # 2: eye
            eng.wait_ge(sem_in, 32)
            v.tensor_reduce(rmax2[:, :, None], aa2_t, op=ALU.max,
                            axis=AX.X).then_inc(sem_dve, 1)         # 3
            eng.wait_ge(sem_dve, 3)
            v.tensor_tensor(ohtmp[:], aa2_t,
                            rmax2[:, :, None].to_broadcast((L, 2, NCLS)),
                            op=ALU.is_ge).then_inc(sem_dve, 1)      # 4
            eng.wait_ge(sem_dve, 4)
            v.tensor_tensor(oh_view, ohtmp[:],
                            mask2_t[:, :, None].to_broadcast((L, 2, NCLS)),
                            op=ALU.mult).then_inc(sem_dve, 1)       # 5
            eng.wait_ge(sem_pe, 2)              # transposes done
            v.tensor_copy(ohT2[:], ohT2_p[:]).then_inc(sem_dve, 1)      # 6
            eng.wait_ge(sem_pe, 3)              # W2 done
            v.tensor_copy(W2[:], w2m4_p[:NCLS, :]).then_inc(sem_dve, 1)  # 7
            eng.wait_ge(sem_d2, 4)              # distances ready
            v.tensor_scalar(
                rcols4[:].rearrange("j (a b) -> j a b", b=RPC),
                acc[:, None, :].to_broadcast((L, 4, RPC)),
                R2, None, ALU.is_lt).then_inc(sem_dve, 1)           # 8
            eng.wait_ge(sem_pe, 4)              # temb2 done
            eng.wait_ge(sem_cold2, 16)          # remb rows 64:128
            v.scalar_tensor_tensor(S_t[:], remb_f_t, temb2_p[:, D:D + 1],
                                   temb2_p[:, :D], ALU.mult,
                                   ALU.add).then_inc(sem_dve, 1)    # 9: S
            eng.wait_ge(sem_pe, 8)              # scm4 matmuls done
            v.tensor_copy(scm4s[:], scm4_p[:]).then_inc(sem_dve, 1)     # 10
            eng.wait_ge(sem_pe, 9)              # m4 done
            v.tensor_tensor(q4[:], w2m4_p[:, :D], remb_o4_t,
                            op=ALU.subtract).then_inc(sem_dve, 1)   # 11: q4
            eng.wait_ge(sem_atomg, 64)
            eng.wait_ge(sem_dve, 11)
            v.tensor_tensor(
                v4[:], q4[:, None, :].to_broadcast((L, 9, D)),
                atom_rep[:],
                op=ALU.subtract).then_inc(sem_dve, 1)               # 12 sub
            eng.wait_ge(sem_dve, 12)
            v.tensor_tensor(
                o4[:], v4[:],
                scm4s[:, :, None].to_broadcast((L, 9, D)),
                op=ALU.mult).then_inc(sem_dve, 1)                   # 13 mul

        # ---------------- PE ----------------
        @block.tensor
        def _(eng):
            t = nc.tensor
            eng.wait_ge(sem_dve, 5)             # one-hots written
            t.transpose(ohT2_p[:, :L], oh_all[:, :RPC],
                        eye_bf[:]).then_inc(sem_pe, 1)              # 1
            t.transpose(ohT2_p[:, L:], oh_all[:, RPC:],
                        eye_bf[:]).then_inc(sem_pe, 1)              # 2
            eng.wait_ge(sem_cold, 16)           # tables are in rows 0:64
            t.matmul(w2m4_p[:NCLS, :], tblTbf_t,
                     atonesbf_t).then_inc(sem_pe, 1)                # 3: W2
            eng.wait_ge(sem_dve, 7)             # ohT2 + W2 copies done
            t.matmul(temb2_p[:], ohT2[:NCLS, :L],
                     W2[:]).then_inc(sem_pe, 1)                     # 4: [temb|cnt]
            for tg in range(4):
                tb, tw = TB[tg], TW[tg]
                t.matmul(scm4_p[32 * tg:32 * (tg + 1), :tw],
                         ohT2[:NCLS, L + 32 * tg:L + 32 * (tg + 1)],
                         tblbf_t[:, tb:tb + tw],
                         tile_position=(0, 32 * tg),
                         ).then_inc(sem_pe, 1)                      # 5-8
            eng.wait_ge(sem_dve, 9)             # S + rcols4 ready
            t.matmul(w2m4_p[:, :D], rcols4[:],
                     S_t[:]).then_inc(sem_pe, 1)                    # 9: m4

    nc.compile()
    return nc


def make_in_maps(aa_pred, residue_embeddings, bb_pred, mask,
                 valid_atom37_mask, atom_embed):
    f32 = lambda x: np.ascontiguousarray(x, dtype=np.float32)
    tbl_sc = f32(valid_atom37_mask[:NCLS, 3:])          # [20, 34]
    atom_sc = f32(atom_embed[3:])                       # [34, 128]
    atomones = np.concatenate(
        [atom_sc, np.ones((NSC, 1), np.float32)], axis=1)
    tblbf = np.ascontiguousarray(
        tbl_sc.astype(ml_dtypes.bfloat16)).view(np.float32)  # [20, 17]
    in_maps = []
    for c in range(NCORES):
        b = c // (NCORES // B)
        r0 = (c % (NCORES // B)) * RPC
        pk = np.zeros((L, PACKW), dtype=np.float32)

        def put(name, arr):
            arr = f32(arr)
            pk[:arr.shape[0], _off[name]:_off[name] + arr.shape[1]] = arr

        put("aa2", np.concatenate(
            [aa_pred[b, :, :NCLS],
             np.tile(aa_pred[b, r0:r0 + RPC, :NCLS], (4, 1))], axis=1))
        put("mask2", np.stack(
            [mask[b], np.tile(mask[b, r0:r0 + RPC], 4)], axis=1))
        put("rembbf", np.ascontiguousarray(
            f32(residue_embeddings[b]).astype(ml_dtypes.bfloat16))
            .view(np.float32))
        put("tblTbf", np.ascontiguousarray(
            tbl_sc.T.astype(ml_dtypes.bfloat16)).view(np.float32))
        aob = np.zeros((NSC, 130), ml_dtypes.bfloat16)
        aob[:, :D + 1] = atomones.astype(ml_dtypes.bfloat16)
        put("atonesbf", np.ascontiguousarray(aob).view(np.float32))
        put("rembo4bf", np.ascontiguousarray(np.tile(
            f32(residue_embeddings[b, r0:r0 + RPC]), (4, 1))
            .astype(ml_dtypes.bfloat16)).view(np.float32))
        put("catT", bb_pred[b, :, 1, :])
        put("cat_ob", np.tile(
            f32(bb_pred[b, r0:r0 + RPC, 1, :]).T.reshape(1, -1), (L, 1)))
        put("tblbf", tblbf)
        in_maps.append({"pack": pk,
                        "atom": atom_sc.astype(ml_dtypes.bfloat16)})
    return in_maps


def gather_out(results):
    chunks = [np.asarray(r["out"]).reshape(RPC, NSC, D) for r in results]
    full = np.concatenate(chunks, axis=0)          # [256, 34, 128]
    return full.reshape(B, L * NSC, D)


def kernel(**inputs) -> np.ndarray:
    nc = build_nc()
    in_maps = make_in_maps(**inputs)
    res = run_bass_kernel_spmd(nc, in_maps, core_ids=list(range(NCORES)))
    return gather_out(res.results)



# revision 32
# speedup vs baseline: 1.3175x; 1.0035x over previous
"""Trainium2 Bass kernel for nn_AllAtomDecoder (gnn_message_passing).

Math: all 34 side-chain atom slots of residue i are placed at CA_i, so the
[A,A] (A = L*34) radius-graph adjacency is a residue-level [L,L] adjacency
R expanded by per-atom validity vm:
    adj[(i,s),(j,t)] = R[i,j] * vm[i,s] * vm[j,t] * (1 - delta_{(i,s),(j,t)})
with R[i,i] = 1 (distance 0 < 8).  Hence
    msg[(i,s),:] = vm[i,s] * (M[i,:] - remb[i,:] - atom_sc[s,:])
where S[j,:] = cnt_j * remb[j,:] + vm[j,:] @ atom_sc   (cnt_j = sum_t vm[j,t])
      M     = R @ S                                    ([L,L] @ [L,D])
With W = tbl_sc @ [atom_sc | 1] ([20, D+1]):  [temb | cnt] = onehot @ W,
so the only PE work is: W, two one-hot transposes, onehot@W, scm4, R@S.
Pairwise distances run on the vector engine as sum_c (ca_j - ca_own)^2
against a host-broadcast row block; the is_lt(64) also expands R columns
4x along partitions for the t-grouped output layout.

Sharding: 8 cores; cores 0-3 own batch 0, cores 4-7 batch 1; each core
computes the residue-level stages for its batch and emits 32 residues
([32, 34*128] f32) of the final output.

Implementation: raw bacc (no TileContext), hand-placed semaphores.  The
[32 res, 34 t, 128 d] output is packed as partition p = 32*tg + l over 4
overlapping t-groups (bases 0/9/17/25, width 9; duplicated columns
compute identical bytes) so the two big DVE ops run 1152 columns on all
128 partitions.  bf16 (exact for one-hot / table data, ~1e-3 rel for
embeddings) makes the PE matmuls single-pass and the big subtract 2x.
Inputs ride in one packed tensor split hot/cold across the two HWDGE
rings (sync + scalar); the atom-embedding broadcast and the distance
chain run on the otherwise idle GPSIMD engine; output DMAs are spread
over all three DMA queues, and their drain is shadowed by the NEFF
epilogue (no engine waits on them; the epilogue outlasts the transfer).
"""

from contextlib import ExitStack

import ml_dtypes
import numpy as np

import concourse.bacc as bacc
import concourse.mybir as mybir
from concourse.bass_utils import run_bass_kernel_spmd

F32 = mybir.dt.float32
BF16 = mybir.dt.bfloat16
ALU = mybir.AluOpType
AX = mybir.AxisListType

B = 2
L = 128          # residues per batch
NCLS = 20        # enabled residue classes (>=20 are argmax-disabled)
NSC = 34         # side-chain atom slots
D = 128          # embedding dim
RPC = 32         # residues per core
NCORES = 8
R2 = 64.0        # RADIUS**2

TB = [0, 9, 17, 25]   # t-group bases (tg1/tg2 and tg2/tg3 overlap by one
TW = [9, 9, 9, 9]     # column; duplicated columns compute identical bytes)

# pack column layout (f32 columns; aa2/mask2 pairs must stay adjacent)
_widths = dict(aa2=2 * NCLS, mask2=2, catT=3, cat_ob=96,       # hot
               rembbf=D // 2, tblTbf=NCLS // 2, atonesbf=65,    # cold
               rembo4bf=D // 2, tblbf=NSC // 2)
HOTW = 2 * NCLS + 2 + 3 + 96
_off = {}
_c = 0
for _name, _w in _widths.items():
    _off[_name] = _c
    _c += _w
PACKW = _c


def build_nc():
    """Build the SPMD per-core Bass graph (identical on all 8 cores)."""
    nc = bacc.Bacc("TRN2", target_bir_lowering=False, debug=False,
                   num_devices=NCORES)

    pack = nc.dram_tensor("pack", [L, PACKW], F32, kind="ExternalInput")
    atom = nc.dram_tensor("atom", [NSC, D], BF16, kind="ExternalInput")
    out = nc.dram_tensor("out", [RPC, NSC * D], F32, kind="ExternalOutput")
    out3 = out[:].rearrange("l (t d) -> l t d", d=D)
    aflat = atom[:].rearrange("t d -> (t d)")

    with ExitStack() as ctx:
        e = ctx.enter_context

        # ---------------- SBUF ----------------
        pk = e(nc.sbuf_tensor([L, PACKW], F32))
        eye_sb = e(nc.sbuf_tensor([L, L], F32))
        eye_bf = e(nc.sbuf_tensor([L, L], BF16))
        oh_all = e(nc.sbuf_tensor([L, 64], BF16))
        ohtmp = e(nc.sbuf_tensor([L, 2, NCLS], F32))
        rmax2 = e(nc.sbuf_tensor([L, 2], F32))
        dx = e(nc.sbuf_tensor([L, 96], F32))
        sq96 = e(nc.sbuf_tensor([L, 96], F32))
        s01 = e(nc.sbuf_tensor([L, RPC], F32))
        acc = e(nc.sbuf_tensor([L, RPC], F32))
        ohT2 = e(nc.sbuf_tensor([RPC, 2 * L], BF16))
        W2 = e(nc.sbuf_tensor([NCLS, D + 1], BF16))
        S_t = e(nc.sbuf_tensor([L, D], BF16))
        rcols4 = e(nc.sbuf_tensor([L, L], BF16))
        scm4s = e(nc.sbuf_tensor([L, 9], BF16))
        q4 = e(nc.sbuf_tensor([L, D], BF16))
        atom_rep = e(nc.sbuf_tensor([L, 9, D], BF16))
        v4 = e(nc.sbuf_tensor([L, 9, D], BF16))
        o4 = e(nc.sbuf_tensor([L, 9, D], F32))

        def pv(name, rows=L):
            return pk[:rows, _off[name]:_off[name] + _widths[name]]


        aa2_t = pv("aa2").rearrange("p (g c) -> p g c", g=2)       # [128,2,20]
        mask2_t = pv("mask2")                                      # [128,2]
        remb_f_t = pv("rembbf").bitcast(BF16)
        tblTbf_t = pv("tblTbf", NSC).bitcast(BF16)              # [34, 20]
        atonesbf_t = pv("atonesbf", NSC).bitcast(BF16)[:, :D + 1]  # [34, 129]
        remb_o4_t = pv("rembo4bf").bitcast(BF16)
        catT_t = pv("catT")
        cat_ob_t = pv("cat_ob").rearrange("p (c m) -> p c m", c=3)
        tblbf_t = pv("tblbf", NCLS).bitcast(BF16)                  # [20,34] bf16
        # one-hot output view: groups at oh_all cols [0:20] and [32:52]
        oh_view = oh_all[:].rearrange("p (g c) -> p g c", g=2)[:, :, :NCLS]

        # ---------------- PSUM (4 banks) ----------------
        ohT2_p = e(nc.psum_tensor([RPC, 2 * L], BF16))   # b0: both transposes
        w2m4_p = e(nc.psum_tensor([L, D + 1], F32))      # b1: W2 rows 0:20 -> m4
        temb2_p = e(nc.psum_tensor([L, D + 1], F32))     # b2
        scm4_p = e(nc.psum_tensor([L, 9], F32))          # b3

        sem_g = e(nc.semaphore("sem_g"))
        sem_d2 = e(nc.semaphore("sem_d2"))
        sem_cold = e(nc.semaphore("sem_cold"))
        sem_cold2 = e(nc.semaphore("sem_cold2"))
        sem_in = e(nc.semaphore("sem_in"))
        sem_atomg = e(nc.semaphore("sem_atomg"))
        sem_outg = e(nc.semaphore("sem_outg"))
        sem_dve = e(nc.semaphore("sem_dve"))
        sem_pe = e(nc.semaphore("sem_pe"))
        sem_out = e(nc.semaphore("sem_out"))

        block = e(nc.Block(no_gpsimd_drain=True))

        # ------- GPSIMD: identity iota, 4 atom DMAs, distance chain -------
        @block.gpsimd
        def _(eng):
            g = nc.gpsimd
            g.iota(eye_sb[:], pattern=[[1, L]], base=0,
                   channel_multiplier=-1,
                   allow_small_or_imprecise_dtypes=True).then_inc(sem_g, 1)
            for tg in range(4):
                tb, tw = TB[tg], TW[tg]
                g.dma_start(
                    atom_rep[32 * tg:32 * (tg + 1), :tw, :]
                    .rearrange("l t d -> l (t d)"),
                    aflat[tb * D:(tb + tw) * D][None, :]
                    .to_broadcast((RPC, tw * D)),
                ).then_inc(sem_atomg, 16)
            eng.wait_ge(sem_in, 32)
            g.tensor_tensor(dx[:].rearrange("p (c m) -> p c m", c=3),
                            cat_ob_t,
                            catT_t[:, :, None].to_broadcast((L, 3, RPC)),
                            op=ALU.subtract).then_inc(sem_d2, 1)    # 1
            eng.wait_ge(sem_d2, 1)
            g.tensor_tensor(sq96[:], dx[:], dx[:],
                            op=ALU.mult).then_inc(sem_d2, 1)        # 2
            eng.wait_ge(sem_d2, 2)
            g.tensor_tensor(s01[:], sq96[:, :RPC], sq96[:, RPC:2 * RPC],
                            op=ALU.add).then_inc(sem_d2, 1)         # 3
            eng.wait_ge(sem_d2, 3)
            g.tensor_tensor(acc[:], s01[:], sq96[:, 2 * RPC:],
                            op=ALU.add).then_inc(sem_d2, 1)         # 4
            eng.wait_ge(sem_dve, 13)            # mul done
            g.dma_start(
                out3[:, TB[3]:TB[3] + 9, :],
                o4[96:, :, :],
            ).then_inc(sem_outg, 16)

        # ---------------- DMA ring 1: sync ----------------
        @block.sync
        def _(eng):
            eng.dma_start(pk[:64, :HOTW], pack[:64, :HOTW]).then_inc(sem_in, 16)
            eng.dma_start(pk[:64, HOTW:], pack[:64, HOTW:]).then_inc(sem_cold, 16)
            eng.wait_ge(sem_dve, 13)            # mul done
            eng.dma_start(
                out3[:, TB[2]:TB[2] + 9, :],
                o4[64:96, :, :],
            ).then_inc(sem_out, 16)


        # ---------------- DMA ring 2: scalar ----------------
        @block.scalar
        def _(eng):
            eng.dma_start(pk[64:, :HOTW], pack[64:, :HOTW]).then_inc(sem_in, 16)
            eng.dma_start(pk[64:, HOTW:], pack[64:, HOTW:]).then_inc(sem_cold2, 16)
            eng.wait_ge(sem_dve, 13)            # mul done
            eng.dma_start(
                out3[:, :2 * 9, :].rearrange("l (g w) d -> g l (w d)", g=2),
                o4[:64, :, :],
            ).then_inc(sem_out, 16)


        # ---------------- DVE ----------------
        @block.vector
        def _(eng):
            v = nc.vector
            v.memset(oh_all[:], 0.0).then_inc(sem_dve, 1)           # 1
            eng.wait_ge(sem_g, 1)
            v.tensor_scalar(eye_bf[:], eye_sb[:], 0.0, None,
                            ALU.is_equal).then_inc(sem_dve, 1)      # 2: eye
            eng.wait_ge(sem_in, 32)
            v.tensor_reduce(rmax2[:, :, None], aa2_t, op=ALU.max,
                            axis=AX.X).then_inc(sem_dve, 1)         # 3
            eng.wait_ge(sem_dve, 3)
            v.tensor_scalar(oh_all[:, :NCLS], aa2_t[:, 0, :],
                            rmax2[:, :1], mask2_t[:, :1],
                            ALU.is_ge, ALU.mult).then_inc(sem_dve, 1)  # 4
            v.tensor_scalar(oh_all[:, 32:32 + NCLS], aa2_t[:, 1, :],
                            rmax2[:, 1:], mask2_t[:, 1:],
                            ALU.is_ge, ALU.mult).then_inc(sem_dve, 1)  # 5
            eng.wait_ge(sem_pe, 2)              # transposes done
            v.tensor_copy(ohT2[:], ohT2_p[:]).then_inc(sem_dve, 1)      # 6
            eng.wait_ge(sem_pe, 3)              # W2 done
            v.tensor_copy(W2[:], w2m4_p[:NCLS, :]).then_inc(sem_dve, 1)  # 7
            eng.wait_ge(sem_d2, 4)              # distances ready
            v.tensor_scalar(
                rcols4[:].rearrange("j (a b) -> j a b", b=RPC),
                acc[:, None, :].to_broadcast((L, 4, RPC)),
                R2, None, ALU.is_lt).then_inc(sem_dve, 1)           # 8
            eng.wait_ge(sem_pe, 4)              # temb2 done
            eng.wait_ge(sem_cold2, 16)          # remb rows 64:128
            v.scalar_tensor_tensor(S_t[:], remb_f_t, temb2_p[:, D:D + 1],
                                   temb2_p[:, :D], ALU.mult,
                                   ALU.add).then_inc(sem_dve, 1)    # 9: S
            eng.wait_ge(sem_pe, 8)              # scm4 matmuls done
            v.tensor_copy(scm4s[:], scm4_p[:]).then_inc(sem_dve, 1)     # 10
            eng.wait_ge(sem_pe, 9)              # m4 done
            v.tensor_tensor(q4[:], w2m4_p[:, :D], remb_o4_t,
                            op=ALU.subtract).then_inc(sem_dve, 1)   # 11: q4
            eng.wait_ge(sem_atomg, 64)
            eng.wait_ge(sem_dve, 11)
            v.tensor_tensor(
                v4[:], q4[:, None, :].to_broadcast((L, 9, D)),
                atom_rep[:],
                op=ALU.subtract).then_inc(sem_dve, 1)               # 12 sub
            eng.wait_ge(sem_dve, 12)
            v.tensor_tensor(
                o4[:], v4[:],
                scm4s[:, :, None].to_broadcast((L, 9, D)),
                op=ALU.mult).then_inc(sem_dve, 1)                   # 13 mul

        # ---------------- PE ----------------
        @block.tensor
        def _(eng):
            t = nc.tensor
            eng.wait_ge(sem_dve, 4)             # full-batch one-hot written
            t.transpose(ohT2_p[:, :L], oh_all[:, :RPC],
                        eye_bf[:]).then_inc(sem_pe, 1)              # 1
            eng.wait_ge(sem_dve, 5)             # own-rows one-hot written
            t.transpose(ohT2_p[:, L:], oh_all[:, RPC:],
                        eye_bf[:]).then_inc(sem_pe, 1)              # 2
            eng.wait_ge(sem_cold, 16)           # tables are in rows 0:64
            t.matmul(w2m4_p[:NCLS, :], tblTbf_t,
                     atonesbf_t).then_inc(sem_pe, 1)                # 3: W2
            eng.wait_ge(sem_dve, 7)             # ohT2 + W2 copies done
            t.matmul(temb2_p[:], ohT2[:NCLS, :L],
                     W2[:]).then_inc(sem_pe, 1)                     # 4: [temb|cnt]
            for tg in range(4):
                tb, tw = TB[tg], TW[tg]
                t.matmul(scm4_p[32 * tg:32 * (tg + 1), :tw],
                         ohT2[:NCLS, L + 32 * tg:L + 32 * (tg + 1)],
                         tblbf_t[:, tb:tb + tw],
                         tile_position=(0, 32 * tg),
                         ).then_inc(sem_pe, 1)                      # 5-8
            eng.wait_ge(sem_dve, 9)             # S + rcols4 ready
            t.matmul(w2m4_p[:, :D], rcols4[:],
                     S_t[:]).then_inc(sem_pe, 1)                    # 9: m4

    nc.compile()
    return nc


def make_in_maps(aa_pred, residue_embeddings, bb_pred, mask,
                 valid_atom37_mask, atom_embed):
    f32 = lambda x: np.ascontiguousarray(x, dtype=np.float32)
    tbl_sc = f32(valid_atom37_mask[:NCLS, 3:])          # [20, 34]
    atom_sc = f32(atom_embed[3:])                       # [34, 128]
    atomones = np.concatenate(
        [atom_sc, np.ones((NSC, 1), np.float32)], axis=1)
    tblbf = np.ascontiguousarray(
        tbl_sc.astype(ml_dtypes.bfloat16)).view(np.float32)  # [20, 17]
    in_maps = []
    for c in range(NCORES):
        b = c // (NCORES // B)
        r0 = (c % (NCORES // B)) * RPC
        pk = np.zeros((L, PACKW), dtype=np.float32)

        def put(name, arr):
            arr = f32(arr)
            pk[:arr.shape[0], _off[name]:_off[name] + arr.shape[1]] = arr

        put("aa2", np.concatenate(
            [aa_pred[b, :, :NCLS],
             np.tile(aa_pred[b, r0:r0 + RPC, :NCLS], (4, 1))], axis=1))
        put("mask2", np.stack(
            [mask[b], np.tile(mask[b, r0:r0 + RPC], 4)], axis=1))
        put("rembbf", np.ascontiguousarray(
            f32(residue_embeddings[b]).astype(ml_dtypes.bfloat16))
            .view(np.float32))
        put("tblTbf", np.ascontiguousarray(
            tbl_sc.T.astype(ml_dtypes.bfloat16)).view(np.float32))
        aob = np.zeros((NSC, 130), ml_dtypes.bfloat16)
        aob[:, :D + 1] = atomones.astype(ml_dtypes.bfloat16)
        put("atonesbf", np.ascontiguousarray(aob).view(np.float32))
        put("rembo4bf", np.ascontiguousarray(np.tile(
            f32(residue_embeddings[b, r0:r0 + RPC]), (4, 1))
            .astype(ml_dtypes.bfloat16)).view(np.float32))
        put("catT", bb_pred[b, :, 1, :])
        put("cat_ob", np.tile(
            f32(bb_pred[b, r0:r0 + RPC, 1, :]).T.reshape(1, -1), (L, 1)))
        put("tblbf", tblbf)
        in_maps.append({"pack": pk,
                        "atom": atom_sc.astype(ml_dtypes.bfloat16)})
    return in_maps


def gather_out(results):
    chunks = [np.asarray(r["out"]).reshape(RPC, NSC, D) for r in results]
    full = np.concatenate(chunks, axis=0)          # [256, 34, 128]
    return full.reshape(B, L * NSC, D)


def kernel(**inputs) -> np.ndarray:
    nc = build_nc()
    in_maps = make_in_maps(**inputs)
    res = run_bass_kernel_spmd(nc, in_maps, core_ids=list(range(NCORES)))
    return gather_out(res.results)



# revision 33
# speedup vs baseline: 1.3358x; 1.0139x over previous
"""Trainium2 Bass kernel for nn_AllAtomDecoder (gnn_message_passing).

Math: all 34 side-chain atom slots of residue i are placed at CA_i, so the
[A,A] (A = L*34) radius-graph adjacency is a residue-level [L,L] adjacency
R expanded by per-atom validity vm:
    adj[(i,s),(j,t)] = R[i,j] * vm[i,s] * vm[j,t] * (1 - delta_{(i,s),(j,t)})
with R[i,i] = 1 (distance 0 < 8).  Hence
    msg[(i,s),:] = vm[i,s] * (M[i,:] - remb[i,:] - atom_sc[s,:])
where S[j,:] = cnt_j * remb[j,:] + vm[j,:] @ atom_sc   (cnt_j = sum_t vm[j,t])
      M     = R @ S                                    ([L,L] @ [L,D])
With W = tbl_sc @ [atom_sc | 1] ([20, D+1]):  [temb | cnt] = onehot @ W,
so the only PE work is: W, two one-hot transposes, onehot@W, scm4, R@S.
Pairwise distances run on the vector engine as sum_c (ca_j - ca_own)^2
against a host-broadcast row block; the is_lt(64) also expands R columns
4x along partitions for the t-grouped output layout.

Sharding: 8 cores; cores 0-3 own batch 0, cores 4-7 batch 1; each core
computes the residue-level stages for its batch and emits 32 residues
([32, 34*128] f32) of the final output.

Implementation: raw bacc (no TileContext), hand-placed semaphores.  The
[32 res, 34 t, 128 d] output is packed as partition p = 32*tg + l over 4
overlapping t-groups (bases 0/9/17/25, width 9; duplicated columns
compute identical bytes) so the two big DVE ops run 1152 columns on all
128 partitions.  bf16 (exact for one-hot / table data, ~1e-3 rel for
embeddings) makes the PE matmuls single-pass and the big subtract 2x.
Inputs ride in one packed tensor split hot/cold across the two HWDGE
rings (sync + scalar); the atom-embedding broadcast and the distance
chain run on the otherwise idle GPSIMD engine; output DMAs are spread
over all three DMA queues, and their drain is shadowed by the NEFF
epilogue (no engine waits on them; the epilogue outlasts the transfer).
"""

from contextlib import ExitStack

import ml_dtypes
import numpy as np

import concourse.bacc as bacc
import concourse.mybir as mybir
from concourse.bass_utils import run_bass_kernel_spmd

F32 = mybir.dt.float32
BF16 = mybir.dt.bfloat16
ALU = mybir.AluOpType
AX = mybir.AxisListType

B = 2
L = 128          # residues per batch
NCLS = 20        # enabled residue classes (>=20 are argmax-disabled)
NSC = 34         # side-chain atom slots
D = 128          # embedding dim
RPC = 32         # residues per core
NCORES = 8
R2 = 64.0        # RADIUS**2

TB = [0, 9, 17, 25]   # t-group bases (tg1/tg2 and tg2/tg3 overlap by one
TW = [9, 9, 9, 9]     # column; duplicated columns compute identical bytes)

# pack column layout (f32 columns; aa2/mask2 pairs must stay adjacent)
_widths = dict(aa2=2 * NCLS, mask2=2, catT=3, cat_ob=96,       # hot
               rembbf=D // 2, tblTbf=NCLS // 2, atonesbf=65,    # cold
               rembo4bf=D // 2, tblbf=NSC // 2)
HOTW = 2 * NCLS + 2 + 3 + 96
_off = {}
_c = 0
for _name, _w in _widths.items():
    _off[_name] = _c
    _c += _w
PACKW = _c


def build_nc():
    """Build the SPMD per-core Bass graph (identical on all 8 cores)."""
    nc = bacc.Bacc("TRN2", target_bir_lowering=False, debug=False,
                   num_devices=NCORES)

    pack = nc.dram_tensor("pack", [L, PACKW], F32, kind="ExternalInput")
    atom = nc.dram_tensor("atom", [NSC, D], BF16, kind="ExternalInput")
    out = nc.dram_tensor("out", [RPC, NSC * D], F32, kind="ExternalOutput")
    out3 = out[:].rearrange("l (t d) -> l t d", d=D)
    aflat = atom[:].rearrange("t d -> (t d)")

    with ExitStack() as ctx:
        e = ctx.enter_context

        # ---------------- SBUF ----------------
        pk = e(nc.sbuf_tensor([L, PACKW], F32))
        eye_sb = e(nc.sbuf_tensor([L, L], F32))
        eye_bf = e(nc.sbuf_tensor([L, L], BF16))
        oh_all = e(nc.sbuf_tensor([L, 64], BF16))
        ohtmp = e(nc.sbuf_tensor([L, 2, NCLS], F32))
        rmax2 = e(nc.sbuf_tensor([L, 2], F32))
        dx = e(nc.sbuf_tensor([L, 96], F32))
        sq96 = e(nc.sbuf_tensor([L, 96], F32))
        s01 = e(nc.sbuf_tensor([L, RPC], F32))
        acc = e(nc.sbuf_tensor([L, RPC], F32))
        ohT2 = e(nc.sbuf_tensor([RPC, 2 * L], BF16))
        W2 = e(nc.sbuf_tensor([NCLS, D + 1], BF16))
        S_t = e(nc.sbuf_tensor([L, D], BF16))
        rcols4 = e(nc.sbuf_tensor([L, L], BF16))
        scm4s = e(nc.sbuf_tensor([L, 9], BF16))
        q4 = e(nc.sbuf_tensor([L, D], BF16))
        atom_rep = e(nc.sbuf_tensor([L, 9, D], BF16))
        v4 = e(nc.sbuf_tensor([L, 9, D], BF16))
        o4 = e(nc.sbuf_tensor([L, 9, D], F32))

        def pv(name, rows=L):
            return pk[:rows, _off[name]:_off[name] + _widths[name]]


        aa2_t = pv("aa2").rearrange("p (g c) -> p g c", g=2)       # [128,2,20]
        mask2_t = pv("mask2")                                      # [128,2]
        remb_f_t = pv("rembbf").bitcast(BF16)
        tblTbf_t = pv("tblTbf", NSC).bitcast(BF16)              # [34, 20]
        atonesbf_t = pv("atonesbf", NSC).bitcast(BF16)[:, :D + 1]  # [34, 129]
        remb_o4_t = pv("rembo4bf").bitcast(BF16)
        catT_t = pv("catT")
        cat_ob_t = pv("cat_ob").rearrange("p (c m) -> p c m", c=3)
        tblbf_t = pv("tblbf", NCLS).bitcast(BF16)                  # [20,34] bf16
        # one-hot output view: groups at oh_all cols [0:20] and [32:52]
        oh_view = oh_all[:].rearrange("p (g c) -> p g c", g=2)[:, :, :NCLS]

        # ---------------- PSUM (4 banks) ----------------
        ohT2_p = e(nc.psum_tensor([RPC, 2 * L], BF16))   # b0: both transposes
        w2m4_p = e(nc.psum_tensor([L, D + 1], F32))      # b1: W2 rows 0:20 -> m4
        temb2_p = e(nc.psum_tensor([L, D + 1], F32))     # b2
        scm4_p = e(nc.psum_tensor([L, 9], F32))          # b3

        sem_g = e(nc.semaphore("sem_g"))
        sem_d2 = e(nc.semaphore("sem_d2"))
        sem_cold = e(nc.semaphore("sem_cold"))
        sem_cold2 = e(nc.semaphore("sem_cold2"))
        sem_in = e(nc.semaphore("sem_in"))
        sem_atomg = e(nc.semaphore("sem_atomg"))
        sem_outg = e(nc.semaphore("sem_outg"))
        sem_dve = e(nc.semaphore("sem_dve"))
        sem_pe = e(nc.semaphore("sem_pe"))
        sem_out = e(nc.semaphore("sem_out"))

        block = e(nc.Block(no_gpsimd_drain=True))

        # ------- GPSIMD: identity iota, 4 atom DMAs, distance chain -------
        @block.gpsimd
        def _(eng):
            g = nc.gpsimd
            g.iota(eye_sb[:], pattern=[[1, L]], base=0,
                   channel_multiplier=-1,
                   allow_small_or_imprecise_dtypes=True).then_inc(sem_g, 1)
            for tg in range(4):
                tb, tw = TB[tg], TW[tg]
                g.dma_start(
                    atom_rep[32 * tg:32 * (tg + 1), :tw, :]
                    .rearrange("l t d -> l (t d)"),
                    aflat[tb * D:(tb + tw) * D][None, :]
                    .to_broadcast((RPC, tw * D)),
                ).then_inc(sem_atomg, 16)
            eng.wait_ge(sem_in, 32)
            g.tensor_tensor(dx[:].rearrange("p (c m) -> p c m", c=3),
                            cat_ob_t,
                            catT_t[:, :, None].to_broadcast((L, 3, RPC)),
                            op=ALU.subtract).then_inc(sem_d2, 1)    # 1
            eng.wait_ge(sem_d2, 1)
            g.tensor_tensor(sq96[:], dx[:], dx[:],
                            op=ALU.mult).then_inc(sem_d2, 1)        # 2
            eng.wait_ge(sem_d2, 2)
            g.tensor_tensor(s01[:], sq96[:, :RPC], sq96[:, RPC:2 * RPC],
                            op=ALU.add).then_inc(sem_d2, 1)         # 3
            eng.wait_ge(sem_d2, 3)
            g.tensor_tensor(acc[:], s01[:], sq96[:, 2 * RPC:],
                            op=ALU.add).then_inc(sem_d2, 1)         # 4
            eng.wait_ge(sem_dve, 13)            # mul done
            g.dma_start(
                out3[:, TB[3]:TB[3] + 9, :],
                o4[96:, :, :],
            ).then_inc(sem_outg, 16)

        # ---------------- DMA ring 1: sync ----------------
        @block.sync
        def _(eng):
            eng.dma_start(pk[:64, :HOTW], pack[:64, :HOTW]).then_inc(sem_in, 16)
            eng.dma_start(pk[:64, HOTW:], pack[:64, HOTW:]).then_inc(sem_cold, 16)
            eng.wait_ge(sem_dve, 13)            # mul done
            eng.dma_start(
                out3[:, TB[2]:TB[2] + 9, :],
                o4[64:96, :, :],
            ).then_inc(sem_out, 16)


        # ---------------- DMA ring 2: scalar ----------------
        @block.scalar
        def _(eng):
            eng.dma_start(pk[64:, :HOTW], pack[64:, :HOTW]).then_inc(sem_in, 16)
            eng.dma_start(pk[64:, HOTW:], pack[64:, HOTW:]).then_inc(sem_cold2, 16)
            eng.wait_ge(sem_dve, 13)            # mul done
            eng.dma_start(
                out3[:, :2 * 9, :].rearrange("l (g w) d -> g l (w d)", g=2),
                o4[:64, :, :],
            ).then_inc(sem_out, 16)


        # ---------------- DVE ----------------
        @block.vector
        def _(eng):
            v = nc.vector
            v.memset(oh_all[:], 0.0).then_inc(sem_dve, 1)           # 1
            eng.wait_ge(sem_g, 1)
            v.tensor_scalar(eye_bf[:], eye_sb[:], 0.0, None,
                            ALU.is_equal).then_inc(sem_dve, 1)      # 2: eye
            eng.wait_ge(sem_in, 32)
            v.tensor_reduce(rmax2[:, :, None], aa2_t, op=ALU.max,
                            axis=AX.X).then_inc(sem_dve, 1)         # 3
            eng.wait_ge(sem_dve, 3)
            v.tensor_scalar(oh_all[:, :NCLS], aa2_t[:, 0, :],
                            rmax2[:, :1], mask2_t[:, :1],
                            ALU.is_ge, ALU.mult).then_inc(sem_dve, 1)  # 4
            v.tensor_scalar(oh_all[:, 32:32 + NCLS], aa2_t[:, 1, :],
                            rmax2[:, 1:], mask2_t[:, 1:],
                            ALU.is_ge, ALU.mult).then_inc(sem_dve, 1)  # 5
            eng.wait_ge(sem_pe, 2)              # transposes done
            v.tensor_copy(ohT2[:], ohT2_p[:]).then_inc(sem_dve, 1)      # 6
            eng.wait_ge(sem_pe, 3)              # W2 done
            v.tensor_copy(W2[:], w2m4_p[:NCLS, :]).then_inc(sem_dve, 1)  # 7
            eng.wait_ge(sem_pe, 4)              # temb2 done
            eng.wait_ge(sem_cold2, 16)          # remb rows 64:128
            v.scalar_tensor_tensor(S_t[:], remb_f_t, temb2_p[:, D:D + 1],
                                   temb2_p[:, :D], ALU.mult,
                                   ALU.add).then_inc(sem_dve, 1)    # 8: S
            eng.wait_ge(sem_d2, 4)              # distances ready
            v.tensor_scalar(
                rcols4[:].rearrange("j (a b) -> j a b", b=RPC),
                acc[:, None, :].to_broadcast((L, 4, RPC)),
                R2, None, ALU.is_lt).then_inc(sem_dve, 1)           # 9: rcols
            eng.wait_ge(sem_pe, 8)              # scm4 matmuls done
            v.tensor_copy(scm4s[:], scm4_p[:]).then_inc(sem_dve, 1)     # 10
            eng.wait_ge(sem_pe, 9)              # m4 done
            v.tensor_tensor(q4[:], w2m4_p[:, :D], remb_o4_t,
                            op=ALU.subtract).then_inc(sem_dve, 1)   # 11: q4
            eng.wait_ge(sem_atomg, 64)
            eng.wait_ge(sem_dve, 11)
            v.tensor_tensor(
                v4[:], q4[:, None, :].to_broadcast((L, 9, D)),
                atom_rep[:],
                op=ALU.subtract).then_inc(sem_dve, 1)               # 12 sub
            eng.wait_ge(sem_dve, 12)
            v.tensor_tensor(
                o4[:], v4[:],
                scm4s[:, :, None].to_broadcast((L, 9, D)),
                op=ALU.mult).then_inc(sem_dve, 1)                   # 13 mul

        # ---------------- PE ----------------
        @block.tensor
        def _(eng):
            t = nc.tensor
            eng.wait_ge(sem_dve, 4)             # full-batch one-hot written
            t.transpose(ohT2_p[:, :L], oh_all[:, :RPC],
                        eye_bf[:]).then_inc(sem_pe, 1)              # 1
            eng.wait_ge(sem_dve, 5)             # own-rows one-hot written
            t.transpose(ohT2_p[:, L:], oh_all[:, RPC:],
                        eye_bf[:]).then_inc(sem_pe, 1)              # 2
            eng.wait_ge(sem_cold, 16)           # tables are in rows 0:64
            t.matmul(w2m4_p[:NCLS, :], tblTbf_t,
                     atonesbf_t).then_inc(sem_pe, 1)                # 3: W2
            eng.wait_ge(sem_dve, 7)             # ohT2 + W2 copies done
            t.matmul(temb2_p[:], ohT2[:NCLS, :L],
                     W2[:]).then_inc(sem_pe, 1)                     # 4: [temb|cnt]
            for tg in range(4):
                tb, tw = TB[tg], TW[tg]
                t.matmul(scm4_p[32 * tg:32 * (tg + 1), :tw],
                         ohT2[:NCLS, L + 32 * tg:L + 32 * (tg + 1)],
                         tblbf_t[:, tb:tb + tw],
                         tile_position=(0, 32 * tg),
                         ).then_inc(sem_pe, 1)                      # 5-8
            eng.wait_ge(sem_dve, 9)             # S + rcols4 ready
            t.matmul(w2m4_p[:, :D], rcols4[:],
                     S_t[:]).then_inc(sem_pe, 1)                    # 9: m4

    nc.compile()
    return nc


def make_in_maps(aa_pred, residue_embeddings, bb_pred, mask,
                 valid_atom37_mask, atom_embed):
    f32 = lambda x: np.ascontiguousarray(x, dtype=np.float32)
    tbl_sc = f32(valid_atom37_mask[:NCLS, 3:])          # [20, 34]
    atom_sc = f32(atom_embed[3:])                       # [34, 128]
    atomones = np.concatenate(
        [atom_sc, np.ones((NSC, 1), np.float32)], axis=1)
    tblbf = np.ascontiguousarray(
        tbl_sc.astype(ml_dtypes.bfloat16)).view(np.float32)  # [20, 17]
    in_maps = []
    for c in range(NCORES):
        b = c // (NCORES // B)
        r0 = (c % (NCORES // B)) * RPC
        pk = np.zeros((L, PACKW), dtype=np.float32)

        def put(name, arr):
            arr = f32(arr)
            pk[:arr.shape[0], _off[name]:_off[name] + arr.shape[1]] = arr

        put("aa2", np.concatenate(
            [aa_pred[b, :, :NCLS],
             np.tile(aa_pred[b, r0:r0 + RPC, :NCLS], (4, 1))], axis=1))
        put("mask2", np.stack(
            [mask[b], np.tile(mask[b, r0:r0 + RPC], 4)], axis=1))
        put("rembbf", np.ascontiguousarray(
            f32(residue_embeddings[b]).astype(ml_dtypes.bfloat16))
            .view(np.float32))
        put("tblTbf", np.ascontiguousarray(
            tbl_sc.T.astype(ml_dtypes.bfloat16)).view(np.float32))
        aob = np.zeros((NSC, 130), ml_dtypes.bfloat16)
        aob[:, :D + 1] = atomones.astype(ml_dtypes.bfloat16)
        put("atonesbf", np.ascontiguousarray(aob).view(np.float32))
        put("rembo4bf", np.ascontiguousarray(np.tile(
            f32(residue_embeddings[b, r0:r0 + RPC]), (4, 1))
            .astype(ml_dtypes.bfloat16)).view(np.float32))
        put("catT", bb_pred[b, :, 1, :])
        put("cat_ob", np.tile(
            f32(bb_pred[b, r0:r0 + RPC, 1, :]).T.reshape(1, -1), (L, 1)))
        put("tblbf", tblbf)
        in_maps.append({"pack": pk,
                        "atom": atom_sc.astype(ml_dtypes.bfloat16)})
    return in_maps


def gather_out(results):
    chunks = [np.asarray(r["out"]).reshape(RPC, NSC, D) for r in results]
    full = np.concatenate(chunks, axis=0)          # [256, 34, 128]
    return full.reshape(B, L * NSC, D)


def kernel(**inputs) -> np.ndarray:
    nc = build_nc()
    in_maps = make_in_maps(**inputs)
    res = run_bass_kernel_spmd(nc, in_maps, core_ids=list(range(NCORES)))
    return gather_out(res.results)

